# revision 60
# baseline (speedup 1.0000x reference)
"""MoE-with-DeepGEMM kernel for 8 Trainium2 NeuronCores.

Problem: M=4096 tokens, D=2048 in-dim, H=2048 out-dim, E=8 experts.
    gate = softmax(x @ gate_w.T + gate_b)            # [M, E], fp32
    y    = (q8(x) @ q8(expert_w[e]).T) -> bf16       # [E, M, H]
    out  = sum_e gate[:, e, None] * y[e].astype(f32) # [M, H]

Strategy: data-parallel over tokens (M). Each of the 8 cores gets
M/8 = 512 tokens, all 8 experts' weights, and computes its output slice
independently — no collectives; the host concatenates the slices.

The PE issue rate is the wall: 1024 DoubleRow expert matmuls plus 24
fp8 gating matmuls at ~220 ns each. The schedule keeps the PE issuing
back-to-back and the HBM supply matched to consumption order:
  - All inputs are HOST-PREARRANGED into flat per-partition layouts so
    DMA descriptor rows are wide; narrow (<2KB) rows crater per-ring
    DMA rate.
  - e0 runs as m-split phases: phase A (mc0,1 x all hc) consumes w0
    strictly k-major as it arrives; phase B (mc2,3) reuses w0 entirely
    from SBUF (zero DMA).
  - Gating is all-fp8 (no bf16 x copy needed — saves 2MB of
    supply-critical head DMA): logit = xq@gw8 + xr8@gw8 + xq@rg8/256
    with fp8-quantized residuals as first-order corrections (the gw
    residual pre-scaled x256 past fp8's subnormal floor, folded back
    by the DVE). Logit error ~0.14%, better than bf16 gating. Because
    its inputs arrive early, the compiler interleaves the gating
    matmuls into e0's supply stalls — they cost ~zero wall-clock.
  - 62 warm-up matmuls bridge the framework preamble (~7.2us) to the
    first data chunk (~14us) so the PE clock (HAM K-level) ramps once
    and never re-throttles (an idle gap >2us drops K 8->4 and costs
    ~3.4us to re-ramp).
  - e0's supply launches as an unchained blast of 256KB/128KB chunks
    (h-half 0 on Sync, h-half 1 on Scalar, xq on GpSimd): each ring
    round-robins packets across in-flight transfers (processor
    sharing), so small early chunks complete first, self-paced by the
    4-deep HW launch slots. Post-w0 transfers (xr, e1..e7 halves) use
    lag-1 dep chains instead: 2 in flight per ring preserves expert
    order at full ring rate (dep-free measured 40us slower; straight
    chains waste ~35% on launch gaps).
  - e0's gate scale is applied in place by the ACT engine during e1;
    e1..e6 combine acc += gate_e * psum as one DVE stt from PSUM.
    Three PSUM-pool padding allocations keep the bank rotation aligned
    (gating block allocates ps_gt + ps_cor + 4 transposes = 6).
  - e7's LAST mc-group runs hc-major (k inner) so each hc tile's
    combine+output-DMA chases the matmul stream instead of serializing
    after it; the final hc is split in half so the last DMA chases the
    last half-stt.

Host-side prep (not device work): fp8 quantize (identical RNE cast the
reference performs), residual quantization for gating, flat layout
packing, bf16->f32 upcast of the output and the final concat.
"""

import os

# The device can linger in a ~20%-downclocked state from a previous
# session (steady matmul cadence 259ns vs 213.5ns at the same HAM
# K-level); resetting cores on runtime init restores full clock.
os.environ.setdefault("NEURON_RT_RESET_CORES", "1")

import numpy as np
import ml_dtypes

import concourse.bacc as bacc
import concourse.bass as bass
import concourse.mybir as mybir
import concourse.tile as tile
from concourse import masks
from concourse.tile import add_dep_helper
from concourse.bass_utils import run_bass_kernel_spmd

M, D, H, E = 4096, 2048, 2048, 8
NCORES = 8
MS = M // NCORES          # tokens per core (512)
MC = MS // 128            # m-chunks of 128 partitions (4)
DS = D // 128             # d-subtiles of 128 (16)
KP = DS // 2              # DoubleRow d-pairs of 256 (8)
NH = 512                  # h columns per matmul (one PSUM bank of f32)
HC = H // NH              # h-chunks (4)
WJ = KP * 4               # wq dim-1 entries per expert (kp x hh x r)
N_WARM = 62               # dummy warm-up matmuls (N=128) for HAM ramp

_NC = None


def _build_program() -> bass.Bass:
    dt = mybir.dt
    nc = bacc.Bacc(None, target_bir_lowering=False)

    # Flat host-prearranged layouts (partition dim first, wide rows):
    #   xq/xf: [p, s, m] with d = s*128 + p
    #   wq:    [p, j, 1024] with j = ((e*KP + kp)*2 + hh)*2 + r,
    #          holding w^T[(2kp+r)*128 + p, hh*1024 + h']
    xq = nc.dram_tensor("xq", [128, DS, MS], dt.float8e4, kind="ExternalInput")
    xr = nc.dram_tensor("xr", [128, DS, MS], dt.float8e4, kind="ExternalInput")
    wq = nc.dram_tensor("wq", [128, E * WJ, 1024], dt.float8e4,
                        kind="ExternalInput")
    gwt = nc.dram_tensor("gwt", [128, DS, 2 * E], dt.float8e4,
                         kind="ExternalInput")
    gb = nc.dram_tensor("gb", [E, 1], dt.float32, kind="ExternalInput")
    out = nc.dram_tensor("out", [MS, H], dt.bfloat16, kind="ExternalOutput")

    with tile.TileContext(nc) as tc, \
            tc.tile_pool(name="const", bufs=1) as constp, \
            tc.tile_pool(name="wpool", bufs=2) as wpool, \
            tc.tile_pool(name="outp", bufs=6) as outp, \
            tc.tile_pool(name="small", bufs=8) as small, \
            tc.tile_pool(name="ps", bufs=8, space="PSUM") as psp:

        # Persistent SBUF tensors.
        xq_sb = constp.tile([128, DS, MS], dt.float8e4, tag="xq")
        xr_sb = constp.tile([128, DS, MS], dt.float8e4, tag="xr")
        gwt_sb = constp.tile([128, DS, 2 * E], dt.float8e4, tag="gwt")
        gb_sb = constp.tile([E, 1], dt.float32, tag="gb")
        id8_sb = constp.tile([E, E], dt.float32, tag="id8")
        gate_sb = constp.tile([128, MC * E], dt.float32, tag="gate")
        lg_sb = constp.tile([E, MS], dt.float32, tag="lg")
        acc_sb = constp.tile([128, MC * H], dt.float32, tag="acc")
        warm_sb = constp.tile([128, 256], dt.bfloat16, tag="warm")

        masks.make_identity(nc, id8_sb[:])
        nc.gpsimd.memset(warm_sb[:], 0.25)

        # PE warm-up: keep the tensor engine busy from t~7.6us (end of
        # the framework preamble) until the first w0/xq chunks land
        # (~9.3us) so the HAM clock ramp never sees an idle gap.
        ps_warm = psp.tile([128, 128], dt.float32, tag="ps", name="ps_warm")
        for _ in range(N_WARM):
            nc.tensor.matmul(
                ps_warm[:], lhsT=warm_sb[:, 0:128], rhs=warm_sb[:, 128:256],
                start=True, stop=True,
            )

        # ---- DMA ladder ----
        # Measured ring facts: each launcher engine feeds its own HW
        # ring (up to 4 transfers in flight, 8 on GpSimd); a ring
        # round-robins packets across in-flight transfers, which is
        # processor sharing — SMALL transfers complete FIRST. The
        # early-window bus is ~260-300 GB/s total. So e0's supply is
        # launched as an unchained blast of small chunks (completion
        # order ~= size order ~= consumption order, self-paced by the
        # 4-deep HW slots), while everything after w0 uses lag-1
        # chains (2 in flight per ring) to stay in prefetch order.
        w_sb0 = wpool.tile([128, WJ, 1024], dt.float8e4, tag="w")
        d_w0 = {}
        for kp in range(KP):
            d_w0[(kp, 0)] = nc.sync.dma_start(
                w_sb0[:, 4 * kp:4 * kp + 2, :], wq[:, 4 * kp:4 * kp + 2, :])
        for kp in range(KP):
            d_w0[(kp, 1)] = nc.scalar.dma_start(
                w_sb0[:, 4 * kp + 2:4 * kp + 4, :],
                wq[:, 4 * kp + 2:4 * kp + 4, :])
        for kp in range(KP):
            nc.gpsimd.dma_start(xq_sb[:, 2 * kp:2 * kp + 2, :],
                                xq[:, 2 * kp:2 * kp + 2, :])
        d_gwt = nc.gpsimd.dma_start(gwt_sb[:], gwt[:, :, :])
        d_gb = nc.gpsimd.dma_start(gb_sb[:], gb[:, :])
        # xr (fp8 residual of x, for the gating correction term) on
        # GpSimd, gated after w0 so it cannot steal supply-critical
        # bandwidth; needed from ~35us (gating block).
        d_xr = []
        dp = None
        for j in range(2):
            dj = nc.gpsimd.dma_start(
                xr_sb[:, j * 8:(j + 1) * 8, :], xr[:, j * 8:(j + 1) * 8, :])
            if j == 0:
                add_dep_helper(dj.ins, d_w0[(KP - 1, 0)].ins,
                               reason="xr after w0 sync ring")
                add_dep_helper(dj.ins, d_w0[(KP - 1, 1)].ins,
                               reason="xr after w0 scalar ring")
            else:
                add_dep_helper(dj.ins, dp.ins, reason="gpsimd ring chain")
            d_xr.append(dj)
            dp = dj

        def rhs_ap(w_sb, kp, hc):
            j = 4 * kp + 2 * (hc // 2)
            q = hc % 2
            return w_sb[:, j:j + 2, q * 512:(q + 1) * 512]

        # Split PSUM->acc copies alternately across ACT and DVE so each
        # phase's copy chain drains twice as fast.
        def copy_out(i, dst, src):
            if i % 2 == 0:
                nc.scalar.copy(dst, src)
            else:
                nc.vector.tensor_copy(dst, src)

        # ---- Expert 0: m-split phases, k-major consumption ----
        # Phase A (mc0,1 x hc0-3) consumes w0 chunks as they arrive;
        # phase B (mc2,3) replays them from SBUF. PSUM -> acc UNSCALED.
        def e0_phase(mcs):
            pss = {
                mc: [psp.tile([128, NH], dt.float32, tag="ps",
                              name=f"ps0_{mc}_{hc}") for hc in range(HC)]
                for mc in mcs
            }
            for kp in range(KP):
                for mc in mcs:
                    lhsT = xq_sb[:, 2 * kp:2 * kp + 2, mc * 128:(mc + 1) * 128]
                    for hc in range(HC):
                        nc.tensor.matmul(
                            pss[mc][hc][:],
                            lhsT=lhsT,
                            rhs=rhs_ap(w_sb0, kp, hc),
                            start=(kp == 0),
                            stop=(kp == KP - 1),
                            perf_mode=mybir.MatmulPerfMode.DoubleRow,
                        )
            i = 0
            for mc in mcs:
                for hc in range(HC):
                    copy_out(i, acc_sb[:, mc * H + hc * NH:mc * H + (hc + 1) * NH],
                             pss[mc][hc][:])
                    i += 1

        e0_phase((0, 1))
        e0_phase((2, 3))

        # ---- Gating matmuls right after phase B. All fp8 DoubleRow,
        # with two first-order corrections so the logits match the f32
        # reference to ~0.14%: logit = xq@gw8 + xr8@gw8 + xq@rg8/256,
        # where gw8=q8(gw), rg8=q8((gw-gw8)*256) (scaled past fp8's
        # subnormal floor), xr8=q8(x-q8(x)). The (xq, gw8) product
        # depends only on early data; the xr product runs last. The
        # scaled rg8 product accumulates in its own PSUM tile and is
        # folded in at 1/256 by the DVE.
        ps_gt = psp.tile([E, MS], dt.float32, tag="ps", name="ps_gt")
        ps_cor = psp.tile([E, MS], dt.float32, tag="ps", name="ps_cor")
        gt_prods = [(xq_sb, 0, ps_gt), (xq_sb, E, ps_cor), (xr_sb, 0, ps_gt)]
        for pi, (a_sb, gcol, ps) in enumerate(gt_prods):
            for kp in range(KP):
                nc.tensor.matmul(
                    ps[:],
                    lhsT=gwt_sb[:, 2 * kp:2 * kp + 2, gcol:gcol + E],
                    rhs=a_sb[:, 2 * kp:2 * kp + 2, :],
                    start=(pi < 2 and kp == 0),
                    stop=(kp == KP - 1 and (pi == 1 or pi == 2)),
                    perf_mode=mybir.MatmulPerfMode.DoubleRow,
                )
        nc.vector.tensor_scalar_add(lg_sb[:], ps_gt[:], gb_sb[:])
        nc.vector.scalar_tensor_tensor(
            lg_sb[:], ps_cor[:], 1.0 / 256.0, lg_sb[:],
            op0=mybir.AluOpType.mult, op1=mybir.AluOpType.add,
        )

        def emit_softmax():
            for mc in range(MC):
                pst = psp.tile([128, E], dt.float32, tag="ps", name=f"ps_t{mc}")
                nc.tensor.transpose(
                    pst[:], lg_sb[:, mc * 128:(mc + 1) * 128], id8_sb[:]
                )
                mx = small.tile([128, 1], dt.float32, tag="sm1")
                nc.vector.tensor_reduce(
                    mx[:], pst[:], mybir.AxisListType.X, mybir.AluOpType.max
                )
                nmx = small.tile([128, 1], dt.float32, tag="sm1")
                nc.vector.tensor_scalar_mul(nmx[:], mx[:], -1.0)
                ex = small.tile([128, E], dt.float32, tag="sm")
                ssum = small.tile([128, 1], dt.float32, tag="sm1")
                nc.scalar.activation(
                    ex[:], pst[:], mybir.ActivationFunctionType.Exp,
                    bias=nmx[:], scale=1.0, accum_out=ssum[:],
                )
                rcp = small.tile([128, 1], dt.float32, tag="sm1")
                nc.vector.reciprocal(rcp[:], ssum[:])
                nc.vector.tensor_scalar_mul(
                    gate_sb[:, mc * E:(mc + 1) * E], ex[:], rcp[:]
                )

        # ---- Experts 1..7: mc-major, DVE combine straight from PSUM ----
        # Output-launch queues: GpSimd only gets early tiles (its
        # end-of-kernel queue drain would otherwise serialize the
        # teardown behind a late transfer).
        out_q = {
            0: [nc.gpsimd, nc.gpsimd, nc.gpsimd, nc.gpsimd],
            1: [nc.scalar, nc.scalar, nc.scalar, nc.scalar],
            2: [nc.sync, nc.sync, nc.gpsimd, nc.scalar],
            3: [nc.scalar, nc.sync, nc.scalar, nc.sync],
        }
        # Post-w0 prefetch: lag-1 chains (2 in flight per ring) keep
        # completions in expert order at full ring rate. Dep-free
        # launches measured 40us slower end-to-end (every expert's
        # weights under processor sharing arrive just-barely-late);
        # straight chains waste ~35% ring rate on launch gaps.
        sync_hist = [d_w0[(6, 0)], d_w0[(7, 0)]]
        scalar_hist = [d_w0[(6, 1)], d_w0[(7, 1)]]
        for e in range(1, E):
            w_sb = wpool.tile([128, WJ, 1024], dt.float8e4, tag="w")
            if e == 1:
                # e1's first half in two 1MB chunks (so early k-groups
                # gate on kp0-1, not a 2MB transfer-end semaphore).
                for c in range(2):
                    dw = nc.sync.dma_start(
                        w_sb[:, 8 * c:8 * c + 8, :],
                        wq[:, (KP + 2 * c) * 4:(KP + 2 * c + 2) * 4, :])
                    add_dep_helper(dw.ins, sync_hist[-2].ins,
                                   reason="sync ring lag-1 chain")
                    sync_hist.append(dw)
            else:
                dw = nc.sync.dma_start(
                    w_sb[:, 0:16, :],
                    wq[:, e * WJ:e * WJ + 16, :])
                add_dep_helper(dw.ins, sync_hist[-2].ins,
                               reason="sync ring lag-1 chain")
                sync_hist.append(dw)
            dw = nc.scalar.dma_start(
                w_sb[:, 16:32, :],
                wq[:, e * WJ + 16:(e + 1) * WJ, :])
            add_dep_helper(dw.ins, scalar_hist[-2].ins,
                           reason="scalar ring lag-1 chain")
            scalar_hist.append(dw)
            for mc in range(MC):
                if e == 1 and mc == 1:
                    # Rotation padding: the softmax block inserted 6
                    # PSUM allocations (ps_gt + ps_cor + 4 transposes),
                    # breaking the 4-slot alternation between mc-groups.
                    # Three pad slots (with DVE memsets emitted AFTER
                    # mc0's combines, so the FIFO has no cycle) realign
                    # the ring: every matmul group again lands on banks
                    # freed a full window earlier.
                    for p in range(3):
                        pad = psp.tile([128, 1], dt.float32, tag="ps",
                                       name=f"ps_pad{p}")
                        nc.vector.memset(pad[:], 0.0)
                msl = slice(mc * 128, (mc + 1) * 128)
                pss = [
                    psp.tile([128, NH], dt.float32, tag="ps", name=f"ps_{e}_{mc}_{i}")
                    for i in range(HC)
                ]
                g_ap = gate_sb[:, mc * E + e:mc * E + e + 1]
                if e == E - 1 and mc == MC - 1:
                    # Final group hc-major: each hc tile's combine+DMA
                    # chases the matmul stream; only the last half-tile
                    # trails the last matmul.
                    for hc in range(HC):
                        for k in range(KP):
                            nc.tensor.matmul(
                                pss[hc][:],
                                lhsT=xq_sb[:, 2 * k:2 * k + 2, msl],
                                rhs=rhs_ap(w_sb, k, hc),
                                start=(k == 0),
                                stop=(k == KP - 1),
                                perf_mode=mybir.MatmulPerfMode.DoubleRow,
                            )
                        a_ap = acc_sb[:, mc * H + hc * NH:mc * H + (hc + 1) * NH]
                        if hc < HC - 1:
                            ot = outp.tile([128, NH], dt.bfloat16, tag="ot")
                            nc.vector.scalar_tensor_tensor(
                                ot[:], pss[hc][:], g_ap, a_ap,
                                op0=mybir.AluOpType.mult,
                                op1=mybir.AluOpType.add,
                            )
                            out_q[mc][hc].dma_start(
                                out[msl, hc * NH:(hc + 1) * NH], ot[:]
                            )
                        else:
            # Final tile: halves so the last DMA chases
                            # the last half-combine.
                            ot = outp.tile([128, NH], dt.bfloat16, tag="ot")
                            for half, q in ((0, nc.scalar), (1, nc.sync)):
                                csl = slice(half * 256, (half + 1) * 256)
                                nc.vector.scalar_tensor_tensor(
                                    ot[:, csl], pss[hc][:, csl], g_ap,
                                    a_ap[:, csl],
                                    op0=mybir.AluOpType.mult,
                                    op1=mybir.AluOpType.add,
                                )
                                q.dma_start(
                                    out[msl, hc * NH + half * 256:
                                        hc * NH + (half + 1) * 256],
                                    ot[:, csl],
                                )
                    continue
                for k in range(KP):
                    lhsT = xq_sb[:, 2 * k:2 * k + 2, msl]
                    for hc in range(HC):
                        nc.tensor.matmul(
                            pss[hc][:],
                            lhsT=lhsT,
                            rhs=rhs_ap(w_sb, k, hc),
                            start=(k == 0),
                            stop=(k == KP - 1),
                            perf_mode=mybir.MatmulPerfMode.DoubleRow,
                        )
                    if e == 1 and mc == 0 and k == 0:
                        # Softmax transposes here: the PE is one k-step
                        # into e1, lg_sb is ready, phase B's copies are
                        # drained — no PE wait.
                        emit_softmax()
                if e == 1:
                    # Deferred e0 gate scale, on ACT (activation Copy
                    # with per-partition scale) so the DVE stays free
                    # for the combines.
                    g0_ap = gate_sb[:, mc * E:mc * E + 1]
                    for hc in range(HC):
                        a_ap = acc_sb[:, mc * H + hc * NH:mc * H + (hc + 1) * NH]
                        nc.scalar.activation(
                            a_ap, a_ap, mybir.ActivationFunctionType.Copy,
                            scale=g0_ap,
                        )
                for hc in range(HC):
                    a_ap = acc_sb[:, mc * H + hc * NH:mc * H + (hc + 1) * NH]
                    if e < E - 1:
                        nc.vector.scalar_tensor_tensor(
                            a_ap, pss[hc][:], g_ap, a_ap,
                            op0=mybir.AluOpType.mult, op1=mybir.AluOpType.add,
                        )
                    else:
                        ot = outp.tile([128, NH], dt.bfloat16, tag="ot")
                        nc.vector.scalar_tensor_tensor(
                            ot[:], pss[hc][:], g_ap, a_ap,
                            op0=mybir.AluOpType.mult, op1=mybir.AluOpType.add,
                        )
                        out_q[mc][hc].dma_start(
                            out[msl, hc * NH:(hc + 1) * NH], ot[:]
                        )

    nc.compile()
    return nc


def _get_nc() -> bass.Bass:
    global _NC
    if _NC is None:
        _NC = _build_program()
    return _NC


def _prep_in_maps(x, gate_w, gate_b, expert_w):
    f8fn = ml_dtypes.float8_e4m3fn
    f8trn = ml_dtypes.float8_e4m3  # same bits as e4m3fn for |v| <= 240

    x = np.asarray(x, dtype=np.float32)
    gate_w = np.asarray(gate_w, dtype=np.float32)
    gate_b = np.asarray(gate_b, dtype=np.float32)
    expert_w = np.asarray(expert_w, dtype=np.float32)

    # x^T: [D, M]; fp8 quantized copy plus the fp8-quantized residual
    # (gating correction term: x ~= q8(x) + q8(x - q8(x))).
    xT = np.ascontiguousarray(x.T)                       # [D, M] f32
    xqT_f8 = xT.astype(f8fn)                             # [D, M] fp8
    xqT = xqT_f8.view(f8trn)
    xrT = (xT - xqT_f8.astype(np.float32)).astype(f8fn).view(f8trn)
    # expert_w [E, H, D] -> w^T per expert [E, D, H], quantized, packed
    # into the flat [128, j, 1024] device layout with
    # j = ((e*KP + kp)*2 + hh)*2 + r and d = (2*kp + r)*128 + p.
    wqT = np.ascontiguousarray(
        expert_w.transpose(0, 2, 1)
    ).astype(f8fn).view(f8trn)                           # [E, D, H]
    wq_flat = np.ascontiguousarray(
        wqT.reshape(E, KP, 2, 128, 2, 1024)
           .transpose(3, 0, 1, 4, 2, 5)
           .reshape(128, E * WJ, 1024)
    )
    # gate_w^T in fp8 plus its fp8-quantized residual, packed
    # [128, DS, 2E]: columns 0:E = q8(gw), E:2E = q8(gw - q8(gw)).
    # The gw residual is ~gw/2048 in magnitude — below fp8's subnormal
    # floor — so it is pre-scaled by 256 (the kernel scales the
    # correction product back by 1/256 when forming the logits).
    gwT = np.ascontiguousarray(gate_w.T)                 # [D, E] f32
    gw8 = gwT.astype(f8fn)
    rg8 = ((gwT - gw8.astype(np.float32)) * 256.0).astype(f8fn)
    gwt_flat = np.ascontiguousarray(
        np.concatenate([gw8.view(f8trn), rg8.view(f8trn)], axis=1)
          .reshape(DS, 128, 2 * E).transpose(1, 0, 2)
    )
    gbb = np.ascontiguousarray(gate_b.reshape(E, 1))

    in_maps = []
    for c in range(NCORES):
        csl = slice(c * MS, (c + 1) * MS)
        xq_c = np.ascontiguousarray(
            xqT[:, csl].reshape(DS, 128, MS).transpose(1, 0, 2))
        xr_c = np.ascontiguousarray(
            xrT[:, csl].reshape(DS, 128, MS).transpose(1, 0, 2))
        in_maps.append({
            "xq": xq_c,
            "xr": xr_c,
            "wq": wq_flat,
            "gwt": gwt_flat,
            "gb": gbb,
        })
    return in_maps


def kernel(x, gate_w, gate_b, expert_w, _trace=False, _trace_kwargs=None):
    nc = _get_nc()
    in_maps = _prep_in_maps(x, gate_w, gate_b, expert_w)
    kw = {}
    if _trace:
        kw["trace"] = True
        kw.update(_trace_kwargs or {})
    res = run_bass_kernel_spmd(nc, in_maps, core_ids=list(range(NCORES)), **kw)
    outp = np.concatenate(
        [np.asarray(res.results[c]["out"]).astype(np.float32)
         for c in range(NCORES)],
        axis=0,
    )
    if _trace:
        return outp, res
    return outp


# revision 61
# speedup vs baseline: 1.0010x; 1.0010x over previous
"""MoE-with-DeepGEMM kernel for 8 Trainium2 NeuronCores.

Problem: M=4096 tokens, D=2048 in-dim, H=2048 out-dim, E=8 experts.
    gate = softmax(x @ gate_w.T + gate_b)            # [M, E], fp32
    y    = (q8(x) @ q8(expert_w[e]).T) -> bf16       # [E, M, H]
    out  = sum_e gate[:, e, None] * y[e].astype(f32) # [M, H]

Strategy: data-parallel over tokens (M). Each of the 8 cores gets
M/8 = 512 tokens, all 8 experts' weights, and computes its output slice
independently — no collectives; the host concatenates the slices.

The PE issue rate is the wall: 1024 DoubleRow expert matmuls plus 24
fp8 gating matmuls at ~220 ns each. The schedule keeps the PE issuing
back-to-back and the HBM supply matched to consumption order:
  - All inputs are HOST-PREARRANGED into flat per-partition layouts so
    DMA descriptor rows are wide; narrow (<2KB) rows crater per-ring
    DMA rate.
  - e0 runs as m-split phases: phase A (mc0,1 x all hc) consumes w0
    strictly k-major as it arrives; phase B (mc2,3) reuses w0 entirely
    from SBUF (zero DMA).
  - Gating is all-fp8 (no bf16 x copy needed — saves 2MB of
    supply-critical head DMA): logit = xq@gw8 + xr8@gw8 + xq@rg8/256
    with fp8-quantized residuals as first-order corrections (the gw
    residual pre-scaled x256 past fp8's subnormal floor, folded back
    by the DVE). Logit error ~0.14%, better than bf16 gating. Because
    its inputs arrive early, the compiler interleaves the gating
    matmuls into e0's supply stalls — they cost ~zero wall-clock.
  - 62 warm-up matmuls bridge the framework preamble (~7.2us) to the
    first data chunk (~14us) so the PE clock (HAM K-level) ramps once
    and never re-throttles (an idle gap >2us drops K 8->4 and costs
    ~3.4us to re-ramp).
  - e0's supply launches as an unchained blast of 256KB/128KB chunks
    (h-half 0 on Sync, h-half 1 on Scalar, xq on GpSimd): each ring
    round-robins packets across in-flight transfers (processor
    sharing), so small early chunks complete first, self-paced by the
    4-deep HW launch slots. Post-w0 transfers (xr, e1..e7 halves) use
    lag-1 dep chains instead: 2 in flight per ring preserves expert
    order at full ring rate (dep-free measured 40us slower; straight
    chains waste ~35% on launch gaps).
  - e0's gate scale is applied in place by the ACT engine during e1;
    e1..e6 combine acc += gate_e * psum as one DVE stt from PSUM.
    Three PSUM-pool padding allocations keep the bank rotation aligned
    (gating block allocates ps_gt + ps_cor + 4 transposes = 6).
  - e7's LAST mc-group runs hc-major (k inner) so each hc tile's
    combine+output-DMA chases the matmul stream instead of serializing
    after it; the final hc is split in half so the last DMA chases the
    last half-stt.

Host-side prep (not device work): fp8 quantize (identical RNE cast the
reference performs), residual quantization for gating, flat layout
packing, bf16->f32 upcast of the output and the final concat.
"""

import os

# The device can linger in a ~20%-downclocked state from a previous
# session (steady matmul cadence 259ns vs 213.5ns at the same HAM
# K-level); resetting cores on runtime init restores full clock.
os.environ.setdefault("NEURON_RT_RESET_CORES", "1")

import numpy as np
import ml_dtypes

import concourse.bacc as bacc
import concourse.bass as bass
import concourse.mybir as mybir
import concourse.tile as tile
from concourse import masks
from concourse.tile import add_dep_helper
from concourse.bass_utils import run_bass_kernel_spmd

M, D, H, E = 4096, 2048, 2048, 8
NCORES = 8
MS = M // NCORES          # tokens per core (512)
MC = MS // 128            # m-chunks of 128 partitions (4)
DS = D // 128             # d-subtiles of 128 (16)
KP = DS // 2              # DoubleRow d-pairs of 256 (8)
NH = 512                  # h columns per matmul (one PSUM bank of f32)
HC = H // NH              # h-chunks (4)
WJ = KP * 4               # wq dim-1 entries per expert (kp x hh x r)
N_WARM = 62               # dummy warm-up matmuls (N=128) for HAM ramp

_NC = None


def _build_program() -> bass.Bass:
    dt = mybir.dt
    nc = bacc.Bacc(None, target_bir_lowering=False)

    # Flat host-prearranged layouts (partition dim first, wide rows):
    #   xq/xf: [p, s, m] with d = s*128 + p
    #   wq:    [p, j, 1024] with j = ((e*KP + kp)*2 + hh)*2 + r,
    #          holding w^T[(2kp+r)*128 + p, hh*1024 + h']
    xq = nc.dram_tensor("xq", [128, DS, MS], dt.float8e4, kind="ExternalInput")
    xr = nc.dram_tensor("xr", [128, DS, MS], dt.float8e4, kind="ExternalInput")
    wq = nc.dram_tensor("wq", [128, E * WJ, 1024], dt.float8e4,
                        kind="ExternalInput")
    gwt = nc.dram_tensor("gwt", [128, DS, 2 * E], dt.float8e4,
                         kind="ExternalInput")
    gb = nc.dram_tensor("gb", [E, 1], dt.float32, kind="ExternalInput")
    out = nc.dram_tensor("out", [MS, H], dt.bfloat16, kind="ExternalOutput")

    with tile.TileContext(nc) as tc, \
            tc.tile_pool(name="const", bufs=1) as constp, \
            tc.tile_pool(name="wpool", bufs=2) as wpool, \
            tc.tile_pool(name="outp", bufs=6) as outp, \
            tc.tile_pool(name="small", bufs=8) as small, \
            tc.tile_pool(name="ps", bufs=8, space="PSUM") as psp:

        # Persistent SBUF tensors.
        xq_sb = constp.tile([128, DS, MS], dt.float8e4, tag="xq")
        xr_sb = constp.tile([128, DS, MS], dt.float8e4, tag="xr")
        gwt_sb = constp.tile([128, DS, 2 * E], dt.float8e4, tag="gwt")
        gb_sb = constp.tile([E, 1], dt.float32, tag="gb")
        id8_sb = constp.tile([E, E], dt.float32, tag="id8")
        gate_sb = constp.tile([128, MC * E], dt.float32, tag="gate")
        lg_sb = constp.tile([E, MS], dt.float32, tag="lg")
        acc_sb = constp.tile([128, MC * H], dt.float32, tag="acc")
        warm_sb = constp.tile([128, 256], dt.bfloat16, tag="warm")

        masks.make_identity(nc, id8_sb[:])
        nc.gpsimd.memset(warm_sb[:], 0.25)

        # PE warm-up: keep the tensor engine busy from t~7.6us (end of
        # the framework preamble) until the first w0/xq chunks land
        # (~9.3us) so the HAM clock ramp never sees an idle gap.
        ps_warm = psp.tile([128, 128], dt.float32, tag="ps", name="ps_warm")
        for _ in range(N_WARM):
            nc.tensor.matmul(
                ps_warm[:], lhsT=warm_sb[:, 0:128], rhs=warm_sb[:, 128:256],
                start=True, stop=True,
            )

        # ---- DMA ladder ----
        # Measured ring facts: each launcher engine feeds its own HW
        # ring (up to 4 transfers in flight, 8 on GpSimd); a ring
        # round-robins packets across in-flight transfers, which is
        # processor sharing — SMALL transfers complete FIRST. The
        # early-window bus is ~260-300 GB/s total. So e0's supply is
        # launched as an unchained blast of small chunks (completion
        # order ~= size order ~= consumption order, self-paced by the
        # 4-deep HW slots), while everything after w0 uses lag-1
        # chains (2 in flight per ring) to stay in prefetch order.
        w_sb0 = wpool.tile([128, WJ, 1024], dt.float8e4, tag="w")
        d_w0 = {}
        for kp in range(KP):
            d_w0[(kp, 0)] = nc.sync.dma_start(
                w_sb0[:, 4 * kp:4 * kp + 2, :], wq[:, 4 * kp:4 * kp + 2, :])
        for kp in range(KP):
            d_w0[(kp, 1)] = nc.scalar.dma_start(
                w_sb0[:, 4 * kp + 2:4 * kp + 4, :],
                wq[:, 4 * kp + 2:4 * kp + 4, :])
        for kp in range(KP):
            nc.gpsimd.dma_start(xq_sb[:, 2 * kp:2 * kp + 2, :],
                                xq[:, 2 * kp:2 * kp + 2, :])
        d_gwt = nc.gpsimd.dma_start(gwt_sb[:], gwt[:, :, :])
        d_gb = nc.gpsimd.dma_start(gb_sb[:], gb[:, :])
        # xr (fp8 residual of x, for the gating correction term) on
        # GpSimd, gated after w0 so it cannot steal supply-critical
        # bandwidth; needed from ~35us (gating block).
        d_xr = []
        dp = None
        for j in range(2):
            dj = nc.gpsimd.dma_start(
                xr_sb[:, j * 8:(j + 1) * 8, :], xr[:, j * 8:(j + 1) * 8, :])
            if j == 0:
                add_dep_helper(dj.ins, d_w0[(KP - 1, 0)].ins,
                               reason="xr after w0 sync ring")
                add_dep_helper(dj.ins, d_w0[(KP - 1, 1)].ins,
                               reason="xr after w0 scalar ring")
            else:
                add_dep_helper(dj.ins, dp.ins, reason="gpsimd ring chain")
            d_xr.append(dj)
            dp = dj

        def rhs_ap(w_sb, kp, hc):
            j = 4 * kp + 2 * (hc // 2)
            q = hc % 2
            return w_sb[:, j:j + 2, q * 512:(q + 1) * 512]

        # Split PSUM->acc copies alternately across ACT and DVE so each
        # phase's copy chain drains twice as fast.
        def copy_out(i, dst, src):
            if i % 2 == 0:
                nc.scalar.copy(dst, src)
            else:
                nc.vector.tensor_copy(dst, src)

        # ---- Expert 0: m-split phases, k-major consumption ----
        # Phase A (mc0,1 x hc0-3) consumes w0 chunks as they arrive;
        # phase B (mc2,3) replays them from SBUF. PSUM -> acc UNSCALED.
        def e0_phase(mcs):
            pss = {
                mc: [psp.tile([128, NH], dt.float32, tag="ps",
                              name=f"ps0_{mc}_{hc}") for hc in range(HC)]
                for mc in mcs
            }
            for kp in range(KP):
                for mc in mcs:
                    lhsT = xq_sb[:, 2 * kp:2 * kp + 2, mc * 128:(mc + 1) * 128]
                    for hc in range(HC):
                        nc.tensor.matmul(
                            pss[mc][hc][:],
                            lhsT=lhsT,
                            rhs=rhs_ap(w_sb0, kp, hc),
                            start=(kp == 0),
                            stop=(kp == KP - 1),
                            perf_mode=mybir.MatmulPerfMode.DoubleRow,
                        )
            i = 0
            for mc in mcs:
                for hc in range(HC):
                    copy_out(i, acc_sb[:, mc * H + hc * NH:mc * H + (hc + 1) * NH],
                             pss[mc][hc][:])
                    i += 1

        e0_phase((0, 1))
        e0_phase((2, 3))

        # ---- Gating matmuls right after phase B. All fp8 DoubleRow,
        # with two first-order corrections so the logits match the f32
        # reference to ~0.14%: logit = xq@gw8 + xr8@gw8 + xq@rg8/256,
        # where gw8=q8(gw), rg8=q8((gw-gw8)*256) (scaled past fp8's
        # subnormal floor), xr8=q8(x-q8(x)). The (xq, gw8) product
        # depends only on early data; the xr product runs last. The
        # scaled rg8 product accumulates in its own PSUM tile and is
        # folded in at 1/256 by the DVE.
        ps_gt = psp.tile([E, MS], dt.float32, tag="ps", name="ps_gt")
        ps_cor = psp.tile([E, MS], dt.float32, tag="ps", name="ps_cor")
        gt_prods = [(xq_sb, 0, ps_gt), (xq_sb, E, ps_cor), (xr_sb, 0, ps_gt)]
        for pi, (a_sb, gcol, ps) in enumerate(gt_prods):
            for kp in range(KP):
                nc.tensor.matmul(
                    ps[:],
                    lhsT=gwt_sb[:, 2 * kp:2 * kp + 2, gcol:gcol + E],
                    rhs=a_sb[:, 2 * kp:2 * kp + 2, :],
                    start=(pi < 2 and kp == 0),
                    stop=(kp == KP - 1 and (pi == 1 or pi == 2)),
                    perf_mode=mybir.MatmulPerfMode.DoubleRow,
                )
        nc.vector.tensor_scalar_add(lg_sb[:], ps_gt[:], gb_sb[:])
        nc.vector.scalar_tensor_tensor(
            lg_sb[:], ps_cor[:], 1.0 / 256.0, lg_sb[:],
            op0=mybir.AluOpType.mult, op1=mybir.AluOpType.add,
        )

        def emit_softmax():
            for mc in range(MC):
                pst = psp.tile([128, E], dt.float32, tag="ps", name=f"ps_t{mc}")
                nc.tensor.transpose(
                    pst[:], lg_sb[:, mc * 128:(mc + 1) * 128], id8_sb[:]
                )
                mx = small.tile([128, 1], dt.float32, tag="sm1")
                nc.vector.tensor_reduce(
                    mx[:], pst[:], mybir.AxisListType.X, mybir.AluOpType.max
                )
                nmx = small.tile([128, 1], dt.float32, tag="sm1")
                nc.vector.tensor_scalar_mul(nmx[:], mx[:], -1.0)
                ex = small.tile([128, E], dt.float32, tag="sm")
                ssum = small.tile([128, 1], dt.float32, tag="sm1")
                nc.scalar.activation(
                    ex[:], pst[:], mybir.ActivationFunctionType.Exp,
                    bias=nmx[:], scale=1.0, accum_out=ssum[:],
                )
                rcp = small.tile([128, 1], dt.float32, tag="sm1")
                nc.vector.reciprocal(rcp[:], ssum[:])
                nc.vector.tensor_scalar_mul(
                    gate_sb[:, mc * E:(mc + 1) * E], ex[:], rcp[:]
                )

        # ---- Experts 1..7: mc-major, DVE combine straight from PSUM ----
        # Output-launch queues: GpSimd only gets early tiles (its
        # end-of-kernel queue drain would otherwise serialize the
        # teardown behind a late transfer).
        out_q = {
            0: [nc.gpsimd, nc.gpsimd, nc.gpsimd, nc.gpsimd],
            1: [nc.scalar, nc.scalar, nc.scalar, nc.scalar],
            2: [nc.sync, nc.sync, nc.gpsimd, nc.scalar],
            3: [nc.scalar, nc.sync, nc.scalar, nc.sync],
        }
        # Post-w0 prefetch: lag-1 chains (2 in flight per ring) keep
        # completions in expert order at full ring rate. Dep-free
        # launches measured 40us slower end-to-end (every expert's
        # weights under processor sharing arrive just-barely-late);
        # straight chains waste ~35% ring rate on launch gaps.
        sync_hist = [d_w0[(6, 0)], d_w0[(7, 0)]]
        scalar_hist = [d_w0[(6, 1)], d_w0[(7, 1)]]
        for e in range(1, E):
            w_sb = wpool.tile([128, WJ, 1024], dt.float8e4, tag="w")
            if e == 1:
                # e1's first half in two 1MB chunks (so early k-groups
                # gate on kp0-1, not a 2MB transfer-end semaphore).
                for c in range(2):
                    dw = nc.sync.dma_start(
                        w_sb[:, 8 * c:8 * c + 8, :],
                        wq[:, (KP + 2 * c) * 4:(KP + 2 * c + 2) * 4, :])
                    add_dep_helper(dw.ins, sync_hist[-2].ins,
                                   reason="sync ring lag-1 chain")
                    sync_hist.append(dw)
            else:
                dw = nc.sync.dma_start(
                    w_sb[:, 0:16, :],
                    wq[:, e * WJ:e * WJ + 16, :])
                add_dep_helper(dw.ins, sync_hist[-2].ins,
                               reason="sync ring lag-1 chain")
                sync_hist.append(dw)
            dw = nc.scalar.dma_start(
                w_sb[:, 16:32, :],
                wq[:, e * WJ + 16:(e + 1) * WJ, :])
            add_dep_helper(dw.ins, scalar_hist[-2].ins,
                           reason="scalar ring lag-1 chain")
            scalar_hist.append(dw)
            for mc in range(MC):
                if e == 1 and mc == 1:
                    # Rotation padding: the softmax block inserted 6
                    # PSUM allocations (ps_gt + ps_cor + 4 transposes),
                    # breaking the 4-slot alternation between mc-groups.
                    # Three pad slots (with DVE memsets emitted AFTER
                    # mc0's combines, so the FIFO has no cycle) realign
                    # the ring: every matmul group again lands on banks
                    # freed a full window earlier.
                    for p in range(3):
                        pad = psp.tile([128, 1], dt.float32, tag="ps",
                                       name=f"ps_pad{p}")
                        nc.vector.memset(pad[:], 0.0)
                msl = slice(mc * 128, (mc + 1) * 128)
                pss = [
                    psp.tile([128, NH], dt.float32, tag="ps", name=f"ps_{e}_{mc}_{i}")
                    for i in range(HC)
                ]
                g_ap = gate_sb[:, mc * E + e:mc * E + e + 1]
                if e == E - 1 and mc == MC - 1:
                    # Final group hc-major: each hc tile's combine+DMA
                    # chases the matmul stream; only the last half-tile
                    # trails the last matmul.
                    for hc in range(HC):
                        for k in range(KP):
                            nc.tensor.matmul(
                                pss[hc][:],
                                lhsT=xq_sb[:, 2 * k:2 * k + 2, msl],
                                rhs=rhs_ap(w_sb, k, hc),
                                start=(k == 0),
                                stop=(k == KP - 1),
                                perf_mode=mybir.MatmulPerfMode.DoubleRow,
                            )
                        a_ap = acc_sb[:, mc * H + hc * NH:mc * H + (hc + 1) * NH]
                        if hc < HC - 1:
                            ot = outp.tile([128, NH], dt.bfloat16, tag="ot")
                            nc.vector.scalar_tensor_tensor(
                                ot[:], pss[hc][:], g_ap, a_ap,
                                op0=mybir.AluOpType.mult,
                                op1=mybir.AluOpType.add,
                            )
                            out_q[mc][hc].dma_start(
                                out[msl, hc * NH:(hc + 1) * NH], ot[:]
                            )
                        else:
            # Final tile: one full combine, then two
                            # partition-halved DMAs on parallel queues
                            # (64 DRAM rows each — transfer time is
                            # row-count-bound, so halving rows halves
                            # the exposed final-transfer latency).
                            ot = outp.tile([128, NH], dt.bfloat16, tag="ot")
                            nc.vector.scalar_tensor_tensor(
                                ot[:], pss[hc][:], g_ap, a_ap,
                                op0=mybir.AluOpType.mult,
                                op1=mybir.AluOpType.add,
                            )
                            for pp, q in ((0, nc.scalar), (1, nc.sync)):
                                mrsl = slice(mc * 128 + pp * 64,
                                             mc * 128 + (pp + 1) * 64)
                                q.dma_start(
                                    out[mrsl, hc * NH:(hc + 1) * NH],
                                    ot[pp * 64:(pp + 1) * 64, :],
                                )
                    continue
                for k in range(KP):
                    lhsT = xq_sb[:, 2 * k:2 * k + 2, msl]
                    for hc in range(HC):
                        nc.tensor.matmul(
                            pss[hc][:],
                            lhsT=lhsT,
                            rhs=rhs_ap(w_sb, k, hc),
                            start=(k == 0),
                            stop=(k == KP - 1),
                            perf_mode=mybir.MatmulPerfMode.DoubleRow,
                        )
                    if e == 1 and mc == 0 and k == 0:
                        # Softmax transposes here: the PE is one k-step
                        # into e1, lg_sb is ready, phase B's copies are
                        # drained — no PE wait.
                        emit_softmax()
                if e == 1:
                    # Deferred e0 gate scale, on ACT (activation Copy
                    # with per-partition scale) so the DVE stays free
                    # for the combines.
                    g0_ap = gate_sb[:, mc * E:mc * E + 1]
                    for hc in range(HC):
                        a_ap = acc_sb[:, mc * H + hc * NH:mc * H + (hc + 1) * NH]
                        nc.scalar.activation(
                            a_ap, a_ap, mybir.ActivationFunctionType.Copy,
                            scale=g0_ap,
                        )
                for hc in range(HC):
                    a_ap = acc_sb[:, mc * H + hc * NH:mc * H + (hc + 1) * NH]
                    if e < E - 1:
                        nc.vector.scalar_tensor_tensor(
                            a_ap, pss[hc][:], g_ap, a_ap,
                            op0=mybir.AluOpType.mult, op1=mybir.AluOpType.add,
                        )
                    else:
                        ot = outp.tile([128, NH], dt.bfloat16, tag="ot")
                        nc.vector.scalar_tensor_tensor(
                            ot[:], pss[hc][:], g_ap, a_ap,
                            op0=mybir.AluOpType.mult, op1=mybir.AluOpType.add,
                        )
                        out_q[mc][hc].dma_start(
                            out[msl, hc * NH:(hc + 1) * NH], ot[:]
                        )

    nc.compile()
    return nc


def _get_nc() -> bass.Bass:
    global _NC
    if _NC is None:
        _NC = _build_program()
    return _NC


def _prep_in_maps(x, gate_w, gate_b, expert_w):
    f8fn = ml_dtypes.float8_e4m3fn
    f8trn = ml_dtypes.float8_e4m3  # same bits as e4m3fn for |v| <= 240

    x = np.asarray(x, dtype=np.float32)
    gate_w = np.asarray(gate_w, dtype=np.float32)
    gate_b = np.asarray(gate_b, dtype=np.float32)
    expert_w = np.asarray(expert_w, dtype=np.float32)

    # x^T: [D, M]; fp8 quantized copy plus the fp8-quantized residual
    # (gating correction term: x ~= q8(x) + q8(x - q8(x))).
    xT = np.ascontiguousarray(x.T)                       # [D, M] f32
    xqT_f8 = xT.astype(f8fn)                             # [D, M] fp8
    xqT = xqT_f8.view(f8trn)
    xrT = (xT - xqT_f8.astype(np.float32)).astype(f8fn).view(f8trn)
    # expert_w [E, H, D] -> w^T per expert [E, D, H], quantized, packed
    # into the flat [128, j, 1024] device layout with
    # j = ((e*KP + kp)*2 + hh)*2 + r and d = (2*kp + r)*128 + p.
    wqT = np.ascontiguousarray(
        expert_w.transpose(0, 2, 1)
    ).astype(f8fn).view(f8trn)                           # [E, D, H]
    wq_flat = np.ascontiguousarray(
        wqT.reshape(E, KP, 2, 128, 2, 1024)
           .transpose(3, 0, 1, 4, 2, 5)
           .reshape(128, E * WJ, 1024)
    )
    # gate_w^T in fp8 plus its fp8-quantized residual, packed
    # [128, DS, 2E]: columns 0:E = q8(gw), E:2E = q8(gw - q8(gw)).
    # The gw residual is ~gw/2048 in magnitude — below fp8's subnormal
    # floor — so it is pre-scaled by 256 (the kernel scales the
    # correction product back by 1/256 when forming the logits).
    gwT = np.ascontiguousarray(gate_w.T)                 # [D, E] f32
    gw8 = gwT.astype(f8fn)
    rg8 = ((gwT - gw8.astype(np.float32)) * 256.0).astype(f8fn)
    gwt_flat = np.ascontiguousarray(
        np.concatenate([gw8.view(f8trn), rg8.view(f8trn)], axis=1)
          .reshape(DS, 128, 2 * E).transpose(1, 0, 2)
    )
    gbb = np.ascontiguousarray(gate_b.reshape(E, 1))

    in_maps = []
    for c in range(NCORES):
        csl = slice(c * MS, (c + 1) * MS)
        xq_c = np.ascontiguousarray(
            xqT[:, csl].reshape(DS, 128, MS).transpose(1, 0, 2))
        xr_c = np.ascontiguousarray(
            xrT[:, csl].reshape(DS, 128, MS).transpose(1, 0, 2))
        in_maps.append({
            "xq": xq_c,
            "xr": xr_c,
            "wq": wq_flat,
            "gwt": gwt_flat,
            "gb": gbb,
        })
    return in_maps


def kernel(x, gate_w, gate_b, expert_w, _trace=False, _trace_kwargs=None):
    nc = _get_nc()
    in_maps = _prep_in_maps(x, gate_w, gate_b, expert_w)
    kw = {}
    if _trace:
        kw["trace"] = True
        kw.update(_trace_kwargs or {})
    res = run_bass_kernel_spmd(nc, in_maps, core_ids=list(range(NCORES)), **kw)
    outp = np.concatenate(
        [np.asarray(res.results[c]["out"]).astype(np.float32)
         for c in range(NCORES)],
        axis=0,
    )
    if _trace:
        return outp, res
    return outp
